# revision 29
# baseline (speedup 1.0000x reference)
"""Trainium2 Bass kernel for nn_EntityEncoder (3-layer dense transformer encoder).

Sharding: data-parallel over batch, 2 batches per core across 8 NeuronCores.
No collectives; each core computes its 2 rows of the [16, 256] output.

Layout strategy per core (BL=2 batches, T=2048 tokens, A=256 features):
  - Everything lives in "T layout" [A(partitions), tokens(free)]: weights as
    stored are exactly the lhsT the PE wants, and the residual/LayerNorm
    stream stays transposed too (LN stats over the feature/partition dim are
    computed with a ones/256 matmul, which also yields mean/var already
    broadcast across partitions).
  - Attention per (batch, head): contraction dim = head dim = 128 exactly;
    softmax without max-subtraction (exp fused into PSUM eviction with
    scale=1/sqrt(D)); denominator via ones[128,128] matmul (gives Z already
    broadcast over partitions); normalize with reciprocal_approx_fast.
  - rstd = reciprocal_approx_fast(sqrt(var+eps)); Sqrt/Square/Identity stay
    within one ACT table set per phase (2 table loads per layer).
  - Matmul operands are float32r (full-rate PE, ~1e-4 rounding) -- produced
    either by declaring DRAM weights as f32r or by casting on PSUM eviction.
"""

import sys

sys.path.insert(0, "/opt/trn_rl_repo")

import numpy as np

B, N, H, D, DEPTH = 16, 1024, 2, 128, 3
A = H * D  # 256
FFN = 1024
EPS = 1e-5
NCORES = 8
BL = B // NCORES  # 2
T = BL * N  # 2048
P = 128
KA = A // P  # 2
MF = FFN // P  # 8
TT = T // P  # 16
NJ = N // P  # 8 key tiles per batch
NCH = T // 512  # 4 chunks of 512 tokens
INV_SQRT_D = 1.0 / float(np.sqrt(D))

_CACHE = {}


def _build(apply_affine):
    import concourse.bacc as bacc
    import concourse.bass as bass
    import concourse.tile as tile
    from concourse import mybir
    from concourse.masks import make_identity

    f32 = mybir.dt.float32
    f32r = mybir.dt.float32r
    AF = mybir.ActivationFunctionType
    OP = mybir.AluOpType

    nc = bacc.Bacc("TRN2", target_bir_lowering=False, debug=False)

    Xd = nc.dram_tensor("X", [T, A], f32, kind="ExternalInput").ap()
    Wqd = nc.dram_tensor("Wq", [DEPTH, A, A], f32r, kind="ExternalInput").ap()
    Wkd = nc.dram_tensor("Wk", [DEPTH, A, A], f32r, kind="ExternalInput").ap()
    Wvd = nc.dram_tensor("Wv", [DEPTH, A, A], f32r, kind="ExternalInput").ap()
    W1d = nc.dram_tensor("W1", [DEPTH, A, FFN], f32r, kind="ExternalInput").ap()
    W2d = nc.dram_tensor("W2", [DEPTH, FFN, A], f32r, kind="ExternalInput").ap()
    bqd = nc.dram_tensor("bq", [DEPTH, A], f32, kind="ExternalInput").ap()
    bkd = nc.dram_tensor("bk", [DEPTH, A], f32, kind="ExternalInput").ap()
    bvd = nc.dram_tensor("bv", [DEPTH, A], f32, kind="ExternalInput").ap()
    b1d = nc.dram_tensor("b1", [DEPTH, FFN], f32, kind="ExternalInput").ap()
    b2d = nc.dram_tensor("b2", [DEPTH, A], f32r, kind="ExternalInput").ap()
    Wcd = nc.dram_tensor("Wc", [A, A], f32r, kind="ExternalInput").ap()
    bcd = nc.dram_tensor("bc", [A], f32, kind="ExternalInput").ap()
    Wed = nc.dram_tensor("We_s", [A, A], f32, kind="ExternalInput").ap()
    bed = nc.dram_tensor("be", [A], f32, kind="ExternalInput").ap()
    if apply_affine:
        lngd = nc.dram_tensor("ln_g", [A], f32, kind="ExternalInput").ap()
        lnbd = nc.dram_tensor("ln_b", [A], f32, kind="ExternalInput").ap()
    outd = nc.dram_tensor("out", [BL, A], f32, kind="ExternalOutput").ap()

    with tile.TileContext(nc) as tc:
        with (
            tc.tile_pool(name="cp", bufs=1) as cp,
            tc.tile_pool(name="wp", bufs=2) as wp,
            tc.tile_pool(name="ap_", bufs=2) as ap_,
            tc.tile_pool(name="sm", bufs=4) as sm,
            tc.tile_pool(name="psa", bufs=4, space="PSUM") as psa,
            tc.tile_pool(name="psu", bufs=2, space="PSUM") as psu,
            tc.tile_pool(name="psz", bufs=2, space="PSUM") as psz,
        ):
            ident = cp.tile([P, P], f32, tag="ident", name="ident")
            make_identity(nc, ident[:])
            ones_f = cp.tile([P, P], f32, tag="ones_f", name="ones_f")
            nc.vector.memset(ones_f[:], 1.0)
            ones128 = cp.tile([P, P], f32r, tag="ones128", name="ones128")
            nc.vector.tensor_copy(ones128[:], ones_f[:])
            odiv_f = cp.tile([P, P], f32, tag="odiv_f", name="odiv_f")
            nc.vector.memset(odiv_f[:], 1.0 / A)
            onesdiv = cp.tile([P, P], f32r, tag="onesdiv", name="onesdiv")
            nc.vector.tensor_copy(onesdiv[:], odiv_f[:])
            orow_f = cp.tile([1, 512], f32, tag="orow_f", name="orow_f")
            nc.vector.memset(orow_f[:], 1.0)
            ones_row = cp.tile([1, 512], f32r, tag="ones_row", name="ones_row")
            nc.vector.tensor_copy(ones_row[:], orow_f[:])
            epst = cp.tile([P, 1], f32, tag="epst", name="epst")
            nc.vector.memset(epst[:], EPS)
            if apply_affine:
                lng = cp.tile([P, KA], f32, tag="lng", name="lng")
                nc.sync.dma_start(out=lng[:], in_=lngd.rearrange("(k p) -> p k", p=P))
                lnb = cp.tile([P, KA], f32, tag="lnb", name="lnb")
                nc.sync.dma_start(out=lnb[:], in_=lnbd.rearrange("(k p) -> p k", p=P))

            def bcast_ap(src):
                return bass.AP(
                    tensor=src.tensor, offset=src.offset, ap=[[0, P]] + list(src.ap)
                )

            # ---- load X (N layout, transient) and transpose into T layout ----
            step_T = [
                ap_.tile([P, T], f32r, tag="step_T", bufs=2, name=f"xT{k}")
                for k in range(KA)
            ]
            for tg in range(4):
                xt = ap_.tile([P, 4, A], f32, tag="xn", bufs=2, name=f"x{tg}")
                nc.sync.dma_start(
                    out=xt[:],
                    in_=Xd[tg * 4 * P : (tg + 1) * 4 * P, :].rearrange(
                        "(t p) a -> p t a", p=P
                    ),
                )
                for t4 in range(4):
                    t = tg * 4 + t4
                    for k in range(KA):
                        pst = psa.tile([P, P], f32, tag="psa", bufs=4, name="pst")
                        nc.tensor.transpose(
                            pst[:], xt[:, t4, k * P : (k + 1) * P], ident[:]
                        )
                        nc.scalar.copy(step_T[k][:, t * P : (t + 1) * P], pst[:])

            for l in range(DEPTH):
                # ---- layer weights (double-buffered tags; DMA prefetches) ----
                wq = wp.tile([P, KA, A], f32r, tag="wq", name=f"wq{l}")
                nc.sync.dma_start(out=wq[:], in_=Wqd[l].rearrange("(k p) n -> p k n", p=P))
                wk = wp.tile([P, KA, A], f32r, tag="wk", name=f"wk{l}")
                nc.sync.dma_start(out=wk[:], in_=Wkd[l].rearrange("(k p) n -> p k n", p=P))
                wv = wp.tile([P, KA, A], f32r, tag="wv", name=f"wv{l}")
                nc.sync.dma_start(out=wv[:], in_=Wvd[l].rearrange("(k p) n -> p k n", p=P))
                w1 = wp.tile([P, KA, FFN], f32r, tag="w1", name=f"w1{l}")
                nc.sync.dma_start(out=w1[:], in_=W1d[l].rearrange("(k p) n -> p k n", p=P))
                w2 = wp.tile([P, MF, A], f32r, tag="w2", name=f"w2{l}")
                nc.sync.dma_start(out=w2[:], in_=W2d[l].rearrange("(k p) n -> p k n", p=P))
                bqs = wp.tile([P, KA], f32, tag="bqs", name=f"bqs{l}")
                nc.sync.dma_start(out=bqs[:], in_=bqd[l].rearrange("(k p) -> p k", p=P))
                bks = wp.tile([P, KA], f32, tag="bks", name=f"bks{l}")
                nc.sync.dma_start(out=bks[:], in_=bkd[l].rearrange("(k p) -> p k", p=P))
                b1s = wp.tile([P, MF], f32, tag="b1s", name=f"b1s{l}")
                nc.sync.dma_start(out=b1s[:], in_=b1d[l].rearrange("(k p) -> p k", p=P))
                bvb = wp.tile([P, A], f32, tag="bvb", name=f"bvb{l}")
                nc.sync.dma_start(out=bvb[:], in_=bcast_ap(bvd[l]))
                b2r = wp.tile([1, A], f32r, tag="b2r", name=f"b2r{l}")
                nc.sync.dma_start(out=b2r[:], in_=b2d[l : l + 1, :])

                # ---- QKV projections ----
                q_T = [
                    ap_.tile([P, T], f32r, tag="q_T", bufs=2, name=f"q{l}_{m}")
                    for m in range(KA)
                ]
                k_T = [
                    ap_.tile([P, T], f32r, tag="k_T", bufs=2, name=f"k{l}_{m}")
                    for m in range(KA)
                ]
                v_sb = [None] * TT
                for c in range(NCH):
                    cs = slice(c * 512, (c + 1) * 512)
                    for m in range(KA):
                        psq = psa.tile([P, 512], f32, tag="psa", bufs=4, name="psq")
                        for k in range(KA):
                            nc.tensor.matmul(
                                psq[:],
                                wq[:, k, m * P : (m + 1) * P],
                                step_T[k][:, cs],
                                start=(k == 0),
                                stop=(k == KA - 1),
                            )
                        nc.scalar.activation(
                            q_T[m][:, cs], psq[:], AF.Identity, bias=bqs[:, m : m + 1]
                        )
                        psk = psa.tile([P, 512], f32, tag="psa", bufs=4, name="psk")
                        for k in range(KA):
                            nc.tensor.matmul(
                                psk[:],
                                wk[:, k, m * P : (m + 1) * P],
                                step_T[k][:, cs],
                                start=(k == 0),
                                stop=(k == KA - 1),
                            )
                        nc.scalar.activation(
                            k_T[m][:, cs], psk[:], AF.Identity, bias=bks[:, m : m + 1]
                        )
                    for t in range(c * 4, (c + 1) * 4):
                        psv = psa.tile([P, A], f32, tag="psa", bufs=4, name="psv")
                        for k in range(KA):
                            nc.tensor.matmul(
                                psv[:],
                                step_T[k][:, t * P : (t + 1) * P],
                                wv[:, k, :],
                                start=(k == 0),
                                stop=(k == KA - 1),
                            )
                        vt = ap_.tile([P, A], f32r, tag="v", bufs=16, name=f"v{l}_{t}")
                        nc.vector.tensor_tensor(
                            out=vt[:], in0=psv[:], in1=bvb[:], op=OP.add
                        )
                        v_sb[t] = vt

                # ---- attention ----
                att_T = [
                    ap_.tile([P, T], f32r, tag="att_T", bufs=2, name=f"att{l}_{h}")
                    for h in range(H)
                ]
                for b in range(BL):
                    for h in range(H):
                        for c in range(2):  # 512-wide query chunks within batch
                            qs = q_T[h][:, b * N + c * 512 : b * N + (c + 1) * 512]
                            es = []
                            for j in range(NJ):
                                pss = psa.tile([P, 512], f32, tag="psa", bufs=4, name="pss")
                                nc.tensor.matmul(
                                    pss[:],
                                    k_T[h][:, b * N + j * P : b * N + (j + 1) * P],
                                    qs,
                                    start=True,
                                    stop=True,
                                )
                                e = ap_.tile([P, 512], f32r, tag="e", bufs=10, name=f"e{j}")
                                nc.scalar.activation(e[:], pss[:], AF.Exp, scale=INV_SQRT_D)
                                es.append(e)
                            pu = psu.tile([P, 512], f32, tag="psu", bufs=2, name="pu")
                            for j in range(NJ):
                                nc.tensor.matmul(
                                    pu[:],
                                    v_sb[b * NJ + j][:, h * P : (h + 1) * P],
                                    es[j][:],
                                    start=(j == 0),
                                    stop=(j == NJ - 1),
                                )
                            pz = psz.tile([P, 512], f32, tag="psz", bufs=2, name="pz")
                            for j in range(NJ):
                                nc.tensor.matmul(
                                    pz[:],
                                    ones128[:],
                                    es[j][:],
                                    start=(j == 0),
                                    stop=(j == NJ - 1),
                                )
                            r = sm.tile([P, 512], f32, tag="lnw", bufs=4, name="r")
                            nc.vector.reciprocal_approx_fast(out=r[:], in_=pz[:])
                            nc.vector.tensor_tensor(
                                out=att_T[h][:, b * N + c * 512 : b * N + (c + 1) * 512],
                                in0=pu[:],
                                in1=r[:],
                                op=OP.mult,
                            )

                # ---- FFN + residual + LayerNorm (all in T layout) ----
                new_T = [
                    ap_.tile([P, T], f32r, tag="step_T", bufs=2, name=f"sT{l}_{k}")
                    for k in range(KA)
                ]
                def emit_ffn_chunk(c):
                    cs = slice(c * 512, (c + 1) * 512)
                    h1 = []
                    for m in range(MF):
                        ps1 = psa.tile([P, 512], f32, tag="psa", bufs=4, name="ps1")
                        for k in range(KA):
                            nc.tensor.matmul(
                                ps1[:],
                                w1[:, k, m * P : (m + 1) * P],
                                att_T[k][:, cs],
                                start=(k == 0),
                                stop=(k == KA - 1),
                            )
                        hm = ap_.tile([P, 512], f32r, tag="h1", bufs=9, name=f"h1_{m}")
                        if m % 2 == 0:
                            nc.scalar.activation(
                                hm[:], ps1[:], AF.Relu, bias=b1s[:, m : m + 1]
                            )
                        else:
                            nc.vector.tensor_scalar(
                                out=hm[:],
                                in0=ps1[:],
                                scalar1=b1s[:, m : m + 1],
                                scalar2=0.0,
                                op0=OP.add,
                                op1=OP.max,
                            )
                        h1.append(hm)
                    # FFN2 in T orientation: roll_T[m] = W2[:,m].T @ H1 + b2 x 1
                    yt = []
                    for m in range(KA):
                        rp = psu.tile([P, 512], f32, tag="psu", bufs=2, name="rp")
                        for k in range(MF):
                            nc.tensor.matmul(
                                rp[:],
                                w2[:, k, m * P : (m + 1) * P],
                                h1[k][:],
                                start=(k == 0),
                                stop=False,
                            )
                        nc.tensor.matmul(
                            rp[:],
                            b2r[0:1, m * P : (m + 1) * P],
                            ones_row[:],
                            start=False,
                            stop=True,
                        )
                        # y = relu(ffn2 + b2) + step  (one fused DVE op)
                        ym = ap_.tile([P, 512], f32r, tag="yT", bufs=6, name=f"y{m}")
                        nc.vector.scalar_tensor_tensor(
                            out=ym[:],
                            in0=rp[:],
                            scalar=0.0,
                            in1=step_T[m][:, cs],
                            op0=OP.max,
                            op1=OP.add,
                        )
                        yt.append(ym)
                    return yt

                def emit_ln_chunk(c, yt):
                    cs = slice(c * 512, (c + 1) * 512)
                    # LN stats via ones/256 matmuls (results broadcast over partitions)
                    mu = psz.tile([P, 512], f32, tag="psz", bufs=2, name="mu")
                    for k in range(KA):
                        nc.tensor.matmul(
                            mu[:], onesdiv[:], yt[k][:], start=(k == 0), stop=(k == KA - 1)
                        )
                    # centered values; squares of them give the variance directly
                    tsubs = []
                    var = psz.tile([P, 512], f32, tag="psz", bufs=2, name="var")
                    for k in range(KA):
                        tsub = sm.tile([P, 512], f32, tag="lnw", bufs=4, name=f"tsub{k}")
                        nc.vector.tensor_tensor(out=tsub[:], in0=yt[k][:], in1=mu[:], op=OP.subtract)
                        tsubs.append(tsub)
                        y2 = ap_.tile([P, 512], f32r, tag="y2", bufs=2, name=f"y2_{k}")
                        nc.scalar.square(y2[:], tsub[:])
                        nc.tensor.matmul(
                            var[:], onesdiv[:], y2[:], start=(k == 0), stop=(k == KA - 1)
                        )
                    sd = sm.tile([P, 512], f32, tag="lnw", bufs=4, name="sd")
                    nc.scalar.activation(sd[:], var[:], AF.Sqrt, bias=epst[:])
                    rstd = sm.tile([P, 512], f32, tag="lnw", bufs=4, name="rstd")
                    nc.vector.reciprocal_approx_fast(out=rstd[:], in_=sd[:])
                    for k in range(KA):
                        if apply_affine:
                            tnm = sm.tile([P, 512], f32, tag="lnw", bufs=4, name="tnm")
                            nc.vector.tensor_tensor(out=tnm[:], in0=tsubs[k][:], in1=rstd[:], op=OP.mult)
                            nc.scalar.activation(
                                new_T[k][:, cs],
                                tnm[:],
                                AF.Identity,
                                bias=lnb[:, k : k + 1],
                                scale=lng[:, k : k + 1],
                            )
                        else:
                            nc.vector.tensor_tensor(
                                out=new_T[k][:, cs], in0=tsubs[k][:], in1=rstd[:], op=OP.mult
                            )

                # Stagger: emit LN(c-1) after FFN(c) so the LN stats matmuls
                # never head-of-line-block the in-order PE queue.
                pend = {}
                for c in range(NCH):
                    pend[c] = emit_ffn_chunk(c)
                    if c >= 1:
                        emit_ln_chunk(c - 1, pend.pop(c - 1))
                emit_ln_chunk(NCH - 1, pend.pop(NCH - 1))
                step_T = new_T

            # ---- final: conv1d(k=1) + relu, mean-pool (via accum_out), linear ----
            wc = wp.tile([P, KA, A], f32r, tag="wq", name="wc")
            nc.sync.dma_start(out=wc[:], in_=Wcd.rearrange("(k p) n -> p k n", p=P))
            bcs = wp.tile([P, KA], f32, tag="bqs", name="bcs")
            nc.sync.dma_start(out=bcs[:], in_=bcd.rearrange("(k p) -> p k", p=P))
            wes = wp.tile([P, KA, A], f32, tag="wk", name="wes")
            nc.sync.dma_start(out=wes[:], in_=Wed.rearrange("(k p) n -> p k n", p=P))
            bes = wp.tile([P, KA], f32, tag="bks", name="bes")
            nc.sync.dma_start(out=bes[:], in_=bed.rearrange("(k p) -> p k", p=P))

            pooled = [
                sm.tile([P, BL], f32, tag="pooled", bufs=2, name=f"pool{k}")
                for k in range(KA)
            ]
            for m in range(KA):
                parts = []
                for c in range(NCH):
                    cs = slice(c * 512, (c + 1) * 512)
                    psc = psa.tile([P, 512], f32, tag="psa", bufs=4, name="psc")
                    for k in range(KA):
                        nc.tensor.matmul(
                            psc[:],
                            wc[:, k, m * P : (m + 1) * P],
                            step_T[k][:, cs],
                            start=(k == 0),
                            stop=(k == KA - 1),
                        )
                    ee = ap_.tile([P, 512], f32r, tag="e", bufs=10, name=f"ee{m}_{c}")
                    pp = sm.tile([P, 1], f32, tag="pp", bufs=8, name=f"pp{m}_{c}")
                    nc.scalar.activation(
                        ee[:], psc[:], AF.Relu, bias=bcs[:, m : m + 1], accum_out=pp[:]
                    )
                    parts.append(pp)
                for b in range(BL):
                    nc.vector.tensor_tensor(
                        out=pooled[m][:, b : b + 1],
                        in0=parts[2 * b][:],
                        in1=parts[2 * b + 1][:],
                        op=OP.add,
                    )
            for mo in range(KA):
                pse = psa.tile([P, BL], f32, tag="psa", bufs=4, name="pse")
                for k in range(KA):
                    nc.tensor.matmul(
                        pse[:],
                        wes[:, k, mo * P : (mo + 1) * P],
                        pooled[k][:],
                        start=(k == 0),
                        stop=(k == KA - 1),
                    )
                enc = sm.tile([P, BL], f32, tag="enc", bufs=2, name=f"enc{mo}")
                nc.scalar.activation(enc[:], pse[:], AF.Relu, bias=bes[:, mo : mo + 1])
                nc.sync.dma_start(
                    out=outd[:, mo * P : (mo + 1) * P].rearrange("b p -> p b"),
                    in_=enc[:],
                )

    nc.compile()
    return nc


def _get_module(apply_affine):
    key = ("m", apply_affine)
    if key not in _CACHE:
        _CACHE[key] = _build(apply_affine)
    return _CACHE[key]


def kernel(
    X,
    Wq,
    bq,
    Wk,
    bk,
    Wv,
    bv,
    W1,
    b1,
    W2,
    b2,
    ln_g,
    ln_b,
    Wc,
    bc,
    We,
    be,
):
    from concourse.bass_utils import run_bass_kernel_spmd

    X = np.asarray(X, dtype=np.float32)
    apply_affine = not (
        np.all(np.asarray(ln_g) == 1.0) and np.all(np.asarray(ln_b) == 0.0)
    )
    nc = _get_module(apply_affine)

    f32 = np.float32
    shared = {
        "Wq": np.ascontiguousarray(Wq, dtype=f32),
        "Wk": np.ascontiguousarray(Wk, dtype=f32),
        "Wv": np.ascontiguousarray(Wv, dtype=f32),
        "W1": np.ascontiguousarray(W1, dtype=f32),
        "W2": np.ascontiguousarray(W2, dtype=f32),
        "bq": np.ascontiguousarray(bq, dtype=f32),
        "bk": np.ascontiguousarray(bk, dtype=f32),
        "bv": np.ascontiguousarray(bv, dtype=f32),
        "b1": np.ascontiguousarray(b1, dtype=f32),
        "b2": np.ascontiguousarray(b2, dtype=f32),
        "Wc": np.ascontiguousarray(Wc, dtype=f32),
        "bc": np.ascontiguousarray(bc, dtype=f32),
        "We_s": np.ascontiguousarray(np.asarray(We, dtype=f32) / np.float32(N)),
        "be": np.ascontiguousarray(be, dtype=f32),
    }
    if apply_affine:
        shared["ln_g"] = np.ascontiguousarray(ln_g, dtype=f32)
        shared["ln_b"] = np.ascontiguousarray(ln_b, dtype=f32)

    in_maps = []
    for c in range(NCORES):
        m = dict(shared)
        m["X"] = np.ascontiguousarray(
            X[c * BL : (c + 1) * BL].reshape(T, A), dtype=f32
        )
        in_maps.append(m)

    try:
        res = run_bass_kernel_spmd(nc, in_maps, core_ids=list(range(NCORES)))
    except Exception:
        # transient NRT/device hiccups have been observed; one retry
        res = run_bass_kernel_spmd(nc, in_maps, core_ids=list(range(NCORES)))
    out = np.concatenate([res.results[c]["out"] for c in range(NCORES)], axis=0)
    return out.astype(np.float32)


# revision 35
# speedup vs baseline: 1.0753x; 1.0753x over previous
"""Trainium2 Bass kernel for nn_EntityEncoder (3-layer dense transformer encoder).

Sharding: data-parallel over batch, 2 batches per core across 8 NeuronCores.
No collectives; each core computes its 2 rows of the [16, 256] output.

Layout strategy per core (BL=2 batches, T=2048 tokens, A=256 features):
  - Everything lives in "T layout" [A(partitions), tokens(free)]: weights as
    stored are exactly the lhsT the PE wants, and the residual/LayerNorm
    stream stays transposed too (LN stats over the feature/partition dim are
    computed with a ones/256 matmul, which also yields mean/var already
    broadcast across partitions).
  - Attention per (batch, head): contraction dim = head dim = 128 exactly;
    softmax without max-subtraction (exp fused into PSUM eviction with
    scale=1/sqrt(D)); denominator via ones[128,128] matmul (gives Z already
    broadcast over partitions); normalize with reciprocal_approx_fast.
  - rstd = reciprocal_approx_fast(sqrt(var+eps)); Sqrt/Square/Identity stay
    within one ACT table set per phase (2 table loads per layer).
  - Matmul operands are float32r (full-rate PE, ~1e-4 rounding) -- produced
    either by declaring DRAM weights as f32r or by casting on PSUM eviction.
"""

import sys

sys.path.insert(0, "/opt/trn_rl_repo")

import numpy as np

B, N, H, D, DEPTH = 16, 1024, 2, 128, 3
A = H * D  # 256
FFN = 1024
EPS = 1e-5
NCORES = 8
BL = B // NCORES  # 2
T = BL * N  # 2048
P = 128
KA = A // P  # 2
MF = FFN // P  # 8
TT = T // P  # 16
NJ = N // P  # 8 key tiles per batch
NCH = T // 512  # 4 chunks of 512 tokens
INV_SQRT_D = 1.0 / float(np.sqrt(D))

_CACHE = {}


def _build(apply_affine, apply_b2):
    import concourse.bacc as bacc
    import concourse.bass as bass
    import concourse.tile as tile
    from concourse import mybir
    from concourse.masks import make_identity

    f32 = mybir.dt.float32
    f32r = mybir.dt.float32r
    AF = mybir.ActivationFunctionType
    OP = mybir.AluOpType

    nc = bacc.Bacc("TRN2", target_bir_lowering=False, debug=False)

    Xd = nc.dram_tensor("X", [T, A], f32, kind="ExternalInput").ap()
    Wqd = nc.dram_tensor("Wq", [DEPTH, A, A], f32r, kind="ExternalInput").ap()
    Wkd = nc.dram_tensor("Wk", [DEPTH, A, A], f32r, kind="ExternalInput").ap()
    Wvd = nc.dram_tensor("Wv", [DEPTH, A, A], f32r, kind="ExternalInput").ap()
    W1d = nc.dram_tensor("W1", [DEPTH, A, FFN], f32r, kind="ExternalInput").ap()
    W2d = nc.dram_tensor("W2", [DEPTH, FFN, A], f32r, kind="ExternalInput").ap()
    bqd = nc.dram_tensor("bq", [DEPTH, A], f32, kind="ExternalInput").ap()
    bkd = nc.dram_tensor("bk", [DEPTH, A], f32, kind="ExternalInput").ap()
    bvd = nc.dram_tensor("bv", [DEPTH, A], f32, kind="ExternalInput").ap()
    b1d = nc.dram_tensor("b1", [DEPTH, FFN], f32, kind="ExternalInput").ap()
    b2d = nc.dram_tensor("b2", [DEPTH, A], f32r, kind="ExternalInput").ap()
    Wcd = nc.dram_tensor("Wc", [A, A], f32r, kind="ExternalInput").ap()
    bcd = nc.dram_tensor("bc", [A], f32, kind="ExternalInput").ap()
    Wed = nc.dram_tensor("We_s", [A, A], f32, kind="ExternalInput").ap()
    bed = nc.dram_tensor("be", [A], f32, kind="ExternalInput").ap()
    if apply_affine:
        lngd = nc.dram_tensor("ln_g", [A], f32, kind="ExternalInput").ap()
        lnbd = nc.dram_tensor("ln_b", [A], f32, kind="ExternalInput").ap()
    outd = nc.dram_tensor("out", [BL, A], f32, kind="ExternalOutput").ap()

    with tile.TileContext(nc) as tc:
        with (
            tc.tile_pool(name="cp", bufs=1) as cp,
            tc.tile_pool(name="wp", bufs=2) as wp,
            tc.tile_pool(name="ap_", bufs=2) as ap_,
            tc.tile_pool(name="sm", bufs=4) as sm,
            tc.tile_pool(name="psa", bufs=4, space="PSUM") as psa,
            tc.tile_pool(name="psu", bufs=2, space="PSUM") as psu,
            tc.tile_pool(name="psz", bufs=2, space="PSUM") as psz,
        ):
            ident = cp.tile([P, P], f32, tag="ident", name="ident")
            make_identity(nc, ident[:])
            ones_f = cp.tile([P, P], f32, tag="ones_f", name="ones_f")
            nc.vector.memset(ones_f[:], 1.0)
            ones128 = cp.tile([P, P], f32r, tag="ones128", name="ones128")
            nc.vector.tensor_copy(ones128[:], ones_f[:])
            odiv_f = cp.tile([P, P], f32, tag="odiv_f", name="odiv_f")
            nc.vector.memset(odiv_f[:], 1.0 / A)
            onesdiv = cp.tile([P, P], f32r, tag="onesdiv", name="onesdiv")
            nc.vector.tensor_copy(onesdiv[:], odiv_f[:])
            orow_f = cp.tile([1, 512], f32, tag="orow_f", name="orow_f")
            nc.vector.memset(orow_f[:], 1.0)
            ones_row = cp.tile([1, 512], f32r, tag="ones_row", name="ones_row")
            nc.vector.tensor_copy(ones_row[:], orow_f[:])
            epst = cp.tile([P, 1], f32, tag="epst", name="epst")
            nc.vector.memset(epst[:], EPS)
            if apply_affine:
                lng = cp.tile([P, KA], f32, tag="lng", name="lng")
                nc.sync.dma_start(out=lng[:], in_=lngd.rearrange("(k p) -> p k", p=P))
                lnb = cp.tile([P, KA], f32, tag="lnb", name="lnb")
                nc.sync.dma_start(out=lnb[:], in_=lnbd.rearrange("(k p) -> p k", p=P))

            def bcast_ap(src):
                return bass.AP(
                    tensor=src.tensor, offset=src.offset, ap=[[0, P]] + list(src.ap)
                )

            # ---- load X (N layout, transient) and transpose into T layout ----
            step_T = [
                ap_.tile([P, T], f32r, tag="step_T", bufs=2, name=f"xT{k}")
                for k in range(KA)
            ]
            for tg in range(4):
                xt = ap_.tile([P, 4, A], f32, tag="xn", bufs=2, name=f"x{tg}")
                nc.sync.dma_start(
                    out=xt[:],
                    in_=Xd[tg * 4 * P : (tg + 1) * 4 * P, :].rearrange(
                        "(t p) a -> p t a", p=P
                    ),
                )
                for t4 in range(4):
                    t = tg * 4 + t4
                    for k in range(KA):
                        pst = psa.tile([P, P], f32, tag="psa", bufs=4, name="pst")
                        nc.tensor.transpose(
                            pst[:], xt[:, t4, k * P : (k + 1) * P], ident[:]
                        )
                        nc.scalar.copy(step_T[k][:, t * P : (t + 1) * P], pst[:])

            for l in range(DEPTH):
                # ---- layer weights (double-buffered tags; DMA prefetches) ----
                wq = wp.tile([P, KA, A], f32r, tag="wq", name=f"wq{l}")
                nc.sync.dma_start(out=wq[:], in_=Wqd[l].rearrange("(k p) n -> p k n", p=P))
                wk = wp.tile([P, KA, A], f32r, tag="wk", name=f"wk{l}")
                nc.sync.dma_start(out=wk[:], in_=Wkd[l].rearrange("(k p) n -> p k n", p=P))
                wv = wp.tile([P, KA, A], f32r, tag="wv", name=f"wv{l}")
                nc.sync.dma_start(out=wv[:], in_=Wvd[l].rearrange("(k p) n -> p k n", p=P))
                w1 = wp.tile([P, KA, FFN], f32r, tag="w1", name=f"w1{l}")
                nc.sync.dma_start(out=w1[:], in_=W1d[l].rearrange("(k p) n -> p k n", p=P))
                w2 = wp.tile([P, MF, A], f32r, tag="w2", name=f"w2{l}")
                nc.sync.dma_start(out=w2[:], in_=W2d[l].rearrange("(k p) n -> p k n", p=P))
                bqs = wp.tile([P, KA], f32, tag="bqs", name=f"bqs{l}")
                nc.sync.dma_start(out=bqs[:], in_=bqd[l].rearrange("(k p) -> p k", p=P))
                bks = wp.tile([P, KA], f32, tag="bks", name=f"bks{l}")
                nc.sync.dma_start(out=bks[:], in_=bkd[l].rearrange("(k p) -> p k", p=P))
                b1s = wp.tile([P, MF], f32, tag="b1s", name=f"b1s{l}")
                nc.sync.dma_start(out=b1s[:], in_=b1d[l].rearrange("(k p) -> p k", p=P))
                bvb = wp.tile([P, A], f32, tag="bvb", name=f"bvb{l}")
                nc.sync.dma_start(out=bvb[:], in_=bcast_ap(bvd[l]))
                b2r = wp.tile([1, A], f32r, tag="b2r", name=f"b2r{l}")
                nc.sync.dma_start(out=b2r[:], in_=b2d[l : l + 1, :])

                # ---- QKV projections ----
                q_T = [
                    ap_.tile([P, T], f32r, tag="q_T", bufs=2, name=f"q{l}_{m}")
                    for m in range(KA)
                ]
                k_T = [
                    ap_.tile([P, T], f32r, tag="k_T", bufs=2, name=f"k{l}_{m}")
                    for m in range(KA)
                ]
                v_sb = [None] * TT
                for c in range(NCH):
                    cs = slice(c * 512, (c + 1) * 512)
                    for m in range(KA):
                        psq = psa.tile([P, 512], f32, tag="psa", bufs=4, name="psq")
                        for k in range(KA):
                            nc.tensor.matmul(
                                psq[:],
                                wq[:, k, m * P : (m + 1) * P],
                                step_T[k][:, cs],
                                start=(k == 0),
                                stop=(k == KA - 1),
                            )
                        nc.scalar.activation(
                            q_T[m][:, cs], psq[:], AF.Identity, bias=bqs[:, m : m + 1]
                        )
                        psk = psa.tile([P, 512], f32, tag="psa", bufs=4, name="psk")
                        for k in range(KA):
                            nc.tensor.matmul(
                                psk[:],
                                wk[:, k, m * P : (m + 1) * P],
                                step_T[k][:, cs],
                                start=(k == 0),
                                stop=(k == KA - 1),
                            )
                        nc.scalar.activation(
                            k_T[m][:, cs], psk[:], AF.Identity, bias=bks[:, m : m + 1]
                        )
                    for t in range(c * 4, (c + 1) * 4):
                        psv = psa.tile([P, A], f32, tag="psa", bufs=4, name="psv")
                        for k in range(KA):
                            nc.tensor.matmul(
                                psv[:],
                                step_T[k][:, t * P : (t + 1) * P],
                                wv[:, k, :],
                                start=(k == 0),
                                stop=(k == KA - 1),
                            )
                        vt = ap_.tile([P, A], f32r, tag="v", bufs=16, name=f"v{l}_{t}")
                        nc.vector.tensor_tensor(
                            out=vt[:], in0=psv[:], in1=bvb[:], op=OP.add
                        )
                        v_sb[t] = vt

                # ---- attention ----
                att_T = [
                    ap_.tile([P, T], f32r, tag="att_T", bufs=2, name=f"att{l}_{h}")
                    for h in range(H)
                ]
                for b in range(BL):
                    for h in range(H):
                        for c in range(2):  # 512-wide query chunks within batch
                            qs = q_T[h][:, b * N + c * 512 : b * N + (c + 1) * 512]
                            es = []
                            for j in range(NJ):
                                pss = psa.tile([P, 512], f32, tag="psa", bufs=4, name="pss")
                                nc.tensor.matmul(
                                    pss[:],
                                    k_T[h][:, b * N + j * P : b * N + (j + 1) * P],
                                    qs,
                                    start=True,
                                    stop=True,
                                )
                                e = ap_.tile([P, 512], f32r, tag="e", bufs=10, name=f"e{j}")
                                nc.scalar.activation(e[:], pss[:], AF.Exp, scale=INV_SQRT_D)
                                es.append(e)
                            pu = psu.tile([P, 512], f32, tag="psu", bufs=2, name="pu")
                            for j in range(NJ):
                                nc.tensor.matmul(
                                    pu[:],
                                    v_sb[b * NJ + j][:, h * P : (h + 1) * P],
                                    es[j][:],
                                    start=(j == 0),
                                    stop=(j == NJ - 1),
                                )
                            pz = psz.tile([P, 512], f32, tag="psz", bufs=2, name="pz")
                            for j in range(NJ):
                                nc.tensor.matmul(
                                    pz[:],
                                    ones128[:],
                                    es[j][:],
                                    start=(j == 0),
                                    stop=(j == NJ - 1),
                                )
                            r = sm.tile([P, 512], f32, tag="lnw", bufs=4, name="r")
                            nc.vector.reciprocal_approx_fast(out=r[:], in_=pz[:])
                            nc.vector.tensor_tensor(
                                out=att_T[h][:, b * N + c * 512 : b * N + (c + 1) * 512],
                                in0=pu[:],
                                in1=r[:],
                                op=OP.mult,
                            )

                # ---- FFN + residual + LayerNorm (all in T layout) ----
                new_T = [
                    ap_.tile([P, T], f32r, tag="step_T", bufs=2, name=f"sT{l}_{k}")
                    for k in range(KA)
                ]
                def emit_ffn_chunk(c):
                    cs = slice(c * 512, (c + 1) * 512)
                    h1 = []
                    for m in range(MF):
                        ps1 = psa.tile([P, 512], f32, tag="psa", bufs=4, name="ps1")
                        for k in range(KA):
                            nc.tensor.matmul(
                                ps1[:],
                                w1[:, k, m * P : (m + 1) * P],
                                att_T[k][:, cs],
                                start=(k == 0),
                                stop=(k == KA - 1),
                            )
                        hm = ap_.tile([P, 512], f32r, tag="h1", bufs=9, name=f"h1_{m}")
                        if m % 2 == 0:
                            nc.scalar.activation(
                                hm[:], ps1[:], AF.Relu, bias=b1s[:, m : m + 1]
                            )
                        else:
                            nc.vector.tensor_scalar(
                                out=hm[:],
                                in0=ps1[:],
                                scalar1=b1s[:, m : m + 1],
                                scalar2=0.0,
                                op0=OP.add,
                                op1=OP.max,
                            )
                        h1.append(hm)
                    # FFN2 in T orientation: roll_T[m] = W2[:,m].T @ H1 + b2 x 1
                    yt = []
                    for m in range(KA):
                        rp = psu.tile([P, 512], f32, tag="psu", bufs=2, name="rp")
                        for k in range(MF):
                            nc.tensor.matmul(
                                rp[:],
                                w2[:, k, m * P : (m + 1) * P],
                                h1[k][:],
                                start=(k == 0),
                                stop=(not apply_b2 and k == MF - 1),
                            )
                        if apply_b2:
                            nc.tensor.matmul(
                                rp[:],
                                b2r[0:1, m * P : (m + 1) * P],
                                ones_row[:],
                                start=False,
                                stop=True,
                            )
                        # y = relu(ffn2 + b2) + step  (one fused DVE op)
                        ym = ap_.tile([P, 512], f32r, tag="yT", bufs=6, name=f"y{m}")
                        nc.vector.scalar_tensor_tensor(
                            out=ym[:],
                            in0=rp[:],
                            scalar=0.0,
                            in1=step_T[m][:, cs],
                            op0=OP.max,
                            op1=OP.add,
                        )
                        yt.append(ym)
                    return yt

                def emit_ln_chunk(c, yt):
                    cs = slice(c * 512, (c + 1) * 512)
                    # LN stats via ones/256 matmuls (results broadcast over partitions)
                    mu = psz.tile([P, 512], f32, tag="psz", bufs=2, name="mu")
                    for k in range(KA):
                        nc.tensor.matmul(
                            mu[:], onesdiv[:], yt[k][:], start=(k == 0), stop=(k == KA - 1)
                        )
                    # centered values; squares of them give the variance directly
                    tsubs = []
                    var = psz.tile([P, 512], f32, tag="psz", bufs=2, name="var")
                    for k in range(KA):
                        tsub = sm.tile([P, 512], f32, tag="lnw", bufs=4, name=f"tsub{k}")
                        nc.vector.tensor_tensor(out=tsub[:], in0=yt[k][:], in1=mu[:], op=OP.subtract)
                        tsubs.append(tsub)
                        y2 = ap_.tile([P, 512], f32r, tag="y2", bufs=2, name=f"y2_{k}")
                        nc.scalar.square(y2[:], tsub[:])
                        nc.tensor.matmul(
                            var[:], onesdiv[:], y2[:], start=(k == 0), stop=(k == KA - 1)
                        )
                    sd = sm.tile([P, 512], f32, tag="lnw", bufs=4, name="sd")
                    nc.scalar.activation(sd[:], var[:], AF.Sqrt, bias=epst[:])
                    rstd = sm.tile([P, 512], f32, tag="lnw", bufs=4, name="rstd")
                    nc.vector.reciprocal_approx_fast(out=rstd[:], in_=sd[:])
                    for k in range(KA):
                        if apply_affine:
                            tnm = sm.tile([P, 512], f32, tag="lnw", bufs=4, name="tnm")
                            nc.vector.tensor_tensor(out=tnm[:], in0=tsubs[k][:], in1=rstd[:], op=OP.mult)
                            nc.scalar.activation(
                                new_T[k][:, cs],
                                tnm[:],
                                AF.Identity,
                                bias=lnb[:, k : k + 1],
                                scale=lng[:, k : k + 1],
                            )
                        else:
                            nc.vector.tensor_tensor(
                                out=new_T[k][:, cs], in0=tsubs[k][:], in1=rstd[:], op=OP.mult
                            )

                # Stagger: emit LN(c-1) after FFN(c) so the LN stats matmuls
                # never head-of-line-block the in-order PE queue.
                pend = {}
                for c in range(NCH):
                    pend[c] = emit_ffn_chunk(c)
                    if c >= 1:
                        emit_ln_chunk(c - 1, pend.pop(c - 1))
                emit_ln_chunk(NCH - 1, pend.pop(NCH - 1))
                step_T = new_T

            # ---- final: conv1d(k=1) + relu, mean-pool (via accum_out), linear ----
            wc = wp.tile([P, KA, A], f32r, tag="wq", name="wc")
            nc.sync.dma_start(out=wc[:], in_=Wcd.rearrange("(k p) n -> p k n", p=P))
            bcs = wp.tile([P, KA], f32, tag="bqs", name="bcs")
            nc.sync.dma_start(out=bcs[:], in_=bcd.rearrange("(k p) -> p k", p=P))
            wes = wp.tile([P, KA, A], f32, tag="wk", name="wes")
            nc.sync.dma_start(out=wes[:], in_=Wed.rearrange("(k p) n -> p k n", p=P))
            bes = wp.tile([P, KA], f32, tag="bks", name="bes")
            nc.sync.dma_start(out=bes[:], in_=bed.rearrange("(k p) -> p k", p=P))

            pooled = [
                sm.tile([P, BL], f32, tag="pooled", bufs=2, name=f"pool{k}")
                for k in range(KA)
            ]
            for m in range(KA):
                parts = []
                for c in range(NCH):
                    cs = slice(c * 512, (c + 1) * 512)
                    psc = psa.tile([P, 512], f32, tag="psa", bufs=4, name="psc")
                    for k in range(KA):
                        nc.tensor.matmul(
                            psc[:],
                            wc[:, k, m * P : (m + 1) * P],
                            step_T[k][:, cs],
                            start=(k == 0),
                            stop=(k == KA - 1),
                        )
                    ee = ap_.tile([P, 512], f32r, tag="e", bufs=10, name=f"ee{m}_{c}")
                    pp = sm.tile([P, 1], f32, tag="pp", bufs=8, name=f"pp{m}_{c}")
                    nc.scalar.activation(
                        ee[:], psc[:], AF.Relu, bias=bcs[:, m : m + 1], accum_out=pp[:]
                    )
                    parts.append(pp)
                for b in range(BL):
                    nc.vector.tensor_tensor(
                        out=pooled[m][:, b : b + 1],
                        in0=parts[2 * b][:],
                        in1=parts[2 * b + 1][:],
                        op=OP.add,
                    )
            for mo in range(KA):
                pse = psa.tile([P, BL], f32, tag="psa", bufs=4, name="pse")
                for k in range(KA):
                    nc.tensor.matmul(
                        pse[:],
                        wes[:, k, mo * P : (mo + 1) * P],
                        pooled[k][:],
                        start=(k == 0),
                        stop=(k == KA - 1),
                    )
                enc = sm.tile([P, BL], f32, tag="enc", bufs=2, name=f"enc{mo}")
                nc.scalar.activation(enc[:], pse[:], AF.Relu, bias=bes[:, mo : mo + 1])
                nc.sync.dma_start(
                    out=outd[:, mo * P : (mo + 1) * P].rearrange("b p -> p b"),
                    in_=enc[:],
                )

    nc.compile()
    return nc


def _get_module(apply_affine, apply_b2):
    key = ("m", apply_affine, apply_b2)
    if key not in _CACHE:
        _CACHE[key] = _build(apply_affine, apply_b2)
    return _CACHE[key]


def kernel(
    X,
    Wq,
    bq,
    Wk,
    bk,
    Wv,
    bv,
    W1,
    b1,
    W2,
    b2,
    ln_g,
    ln_b,
    Wc,
    bc,
    We,
    be,
):
    from concourse.bass_utils import run_bass_kernel_spmd

    X = np.asarray(X, dtype=np.float32)
    apply_affine = not (
        np.all(np.asarray(ln_g) == 1.0) and np.all(np.asarray(ln_b) == 0.0)
    )
    apply_b2 = bool(np.any(np.asarray(b2)))
    nc = _get_module(apply_affine, apply_b2)

    f32 = np.float32
    shared = {
        "Wq": np.ascontiguousarray(Wq, dtype=f32),
        "Wk": np.ascontiguousarray(Wk, dtype=f32),
        "Wv": np.ascontiguousarray(Wv, dtype=f32),
        "W1": np.ascontiguousarray(W1, dtype=f32),
        "W2": np.ascontiguousarray(W2, dtype=f32),
        "bq": np.ascontiguousarray(bq, dtype=f32),
        "bk": np.ascontiguousarray(bk, dtype=f32),
        "bv": np.ascontiguousarray(bv, dtype=f32),
        "b1": np.ascontiguousarray(b1, dtype=f32),
        "b2": np.ascontiguousarray(b2, dtype=f32),
        "Wc": np.ascontiguousarray(Wc, dtype=f32),
        "bc": np.ascontiguousarray(bc, dtype=f32),
        "We_s": np.ascontiguousarray(np.asarray(We, dtype=f32) / np.float32(N)),
        "be": np.ascontiguousarray(be, dtype=f32),
    }
    if apply_affine:
        shared["ln_g"] = np.ascontiguousarray(ln_g, dtype=f32)
        shared["ln_b"] = np.ascontiguousarray(ln_b, dtype=f32)

    in_maps = []
    for c in range(NCORES):
        m = dict(shared)
        m["X"] = np.ascontiguousarray(
            X[c * BL : (c + 1) * BL].reshape(T, A), dtype=f32
        )
        in_maps.append(m)

    try:
        res = run_bass_kernel_spmd(nc, in_maps, core_ids=list(range(NCORES)))
    except Exception:
        # transient NRT/device hiccups have been observed; one retry
        res = run_bass_kernel_spmd(nc, in_maps, core_ids=list(range(NCORES)))
    out = np.concatenate([res.results[c]["out"] for c in range(NCORES)], axis=0)
    return out.astype(np.float32)
